# revision 20
# baseline (speedup 1.0000x reference)
"""CantorAttention Trainium2 kernel — block-sparse banded attention.

Problem (hardcoded): B=2, S=2048, DIM=512, H=8 heads, D=64, K=64 routes.
  qkv = x @ w_qkv + b_qkv ; per-head sparse attention over routes[q, :] ;
  out = attn_out @ w_out + b_out.

Sharding (8 cores): core i handles batch i//4, heads (2*(i%4), 2*(i%4)+1).
Host gathers: final[b] = sum of the 4 partials of batch b + b_out.

Key idea: routes are k-NN in Cantor-coordinate space. A spectral
seriation of the route graph (host-side) finds a permutation of
positions under which the route matrix is a narrow band: every 128-query
tile's routes fall in a ~229-key window => 2 unaligned 128-key slices.
Attention is computed DENSE per (qtile, slice) block with a
multiplicative count-mask (exact softmax semantics, duplicates
included), skipping everything outside the band: ~5.6x less score/PV/
exp work than full dense.

Softmax denominator: V_aug = [V | ones-col] per slice, so the PV
matmul's row 64 accumulates sum_k pm[k,q] = denominator. den is
replicated across partitions with a tiny selector matmul, reciprocated
and multiplied in per-2-qtile batches.

k-bias is dropped entirely (softmax is invariant to per-query score
shifts); q-bias and the 1/sqrt(D) scale are folded host-side into wq/bq.
"""

import numpy as np
import ml_dtypes

import concourse.bass as bass
import concourse.bacc as bacc
import concourse.mybir as mybir
import concourse.tile as tile
from concourse.bass_utils import run_bass_kernel_spmd
from concourse.masks import make_identity

BF16 = mybir.dt.bfloat16
F32 = mybir.dt.float32
NPBF16 = ml_dtypes.bfloat16
FP8 = mybir.dt.float8e4
NPFP8 = ml_dtypes.float8_e4m3

B = 2
S = 2048
DIM = 512
H = 8
D = 64
KR = 64
SCALE = 0.125

P = 128
NQT = S // P      # 16 query tiles
NC4 = DIM // P    # 4 contraction chunks
QC = 512          # phase-1 column chunk
VA = D + 2        # V_aug block stride (64 V + ones col + pad)

_CACHE = {}


def _plan_windows(routes):
    """Host: permutation + per-qtile key-slice offsets from routes alone."""
    routes = np.asarray(routes)
    s = routes.shape[0]
    x = np.arange(s, dtype=np.float64)
    for _ in range(60):
        x = x[routes].mean(1)
        x -= x.mean()
        n = np.linalg.norm(x)
        if n > 0:
            x /= n
    perm = np.argsort(x, kind="stable").astype(np.int64)
    inv = np.empty(s, np.int64)
    inv[perm] = np.arange(s)
    rk = inv[routes[perm]]  # routes in sorted space
    slices = []
    for t in range(s // P):
        r = rk[t * P:(t + 1) * P]
        lo, hi = int(r.min()), int(r.max())
        n_sl = max(2, int(np.ceil((hi - lo + 1) / P)))
        w0 = min(max(0, lo), s - n_sl * P)
        slices.append([w0 + j * P for j in range(n_sl)])
    return perm, inv, rk, slices


def build_nc(slices):
    key = tuple(tuple(s) for s in slices)
    if key in _CACHE:
        return _CACHE[key]
    nsl = [len(s) for s in slices]          # slices per qtile (>=2)
    tot_sl = sum(nsl)                       # total slice count
    sl_base = np.cumsum([0] + nsl).tolist() # block index base per qtile

    nc = bacc.Bacc(
        "TRN2",
        target_bir_lowering=False,
        debug=False,
        num_devices=8,
    )

    xt_d = nc.dram_tensor("xt", [P, NC4 * S], BF16, kind="ExternalInput").ap()
    # qkv weights packed: wq | wk | wv (4x128 cols each)
    w8_d = nc.dram_tensor("w8", [P, 3 * NC4 * P], BF16,
                          kind="ExternalInput").ap()
    wo_d = nc.dram_tensor("wo", [P, DIM], BF16, kind="ExternalInput").ap()
    bqv_d = nc.dram_tensor("bqv", [P, 2], F32, kind="ExternalInput").ap()
    # mask: per qtile, per slice: [128k, 128q] blocks (shared by both heads)
    msk_d = nc.dram_tensor("msk", [P, tot_sl * P], BF16,
                           kind="ExternalInput").ap()
    out_d = nc.dram_tensor("out", [S, DIM], BF16, kind="ExternalOutput").ap()

    with tile.TileContext(nc) as tc:
        with tc.tile_pool(name="persist", bufs=1) as pp:
            ident = pp.tile([P, P], BF16, tag="ident")
            make_identity(nc, ident[:])

            w8_sb = pp.tile([P, 3 * NC4 * P], BF16, tag="w8")
            w_sb = {n: w8_sb[:, i * NC4 * P:(i + 1) * NC4 * P]
                    for i, n in enumerate(("q", "k", "v"))}
            wo_t = pp.tile([P, DIM], BF16, tag="wo")
            wo_sb = wo_t[:]
            bqv_sb = pp.tile([P, 2], F32, tag="bqv")

            xt_sb = pp.tile([P, NC4 * S], BF16, tag="xt")
            msk_sb = pp.tile([P, tot_sl * P], BF16, tag="msk")
            mw = tot_sl * P
            mh = (tot_sl // 2) * P
            NW = NC4 * P

            def dma_x(qc):
                nc.sync.dma_start(
                    out=xt_sb[:].rearrange("p (c s) -> p c s", c=NC4)[
                        :, :, qc * QC:(qc + 1) * QC],
                    in_=xt_d[:, :].rearrange("p (c s) -> p c s", c=NC4)[
                        :, :, qc * QC:(qc + 1) * QC])

            # ordered so the first phase-1 matmul (q, chunk 0) unblocks
            # as early as possible; masks land before the attention loop
            nc.sync.dma_start(out=w8_sb[:, 0:NW], in_=w8_d[:, 0:NW])
            nc.sync.dma_start(out=bqv_sb[:], in_=bqv_d[:, :])
            nc.sync.dma_start(out=xt_sb[:, 0:S], in_=xt_d[:, 0:S])
            nc.sync.dma_start(out=w8_sb[:, NW:3 * NW], in_=w8_d[:, NW:3 * NW])
            nc.sync.dma_start(out=xt_sb[:, S:2 * S], in_=xt_d[:, S:2 * S])
            nc.sync.dma_start(out=msk_sb[:, 0:mh], in_=msk_d[:, 0:mh])
            nc.sync.dma_start(out=xt_sb[:, 2 * S:4 * S],
                              in_=xt_d[:, 2 * S:4 * S])
            nc.sync.dma_start(out=wo_t[:], in_=wo_d[:, :])
            nc.sync.dma_start(out=msk_sb[:, mh:mw], in_=msk_d[:, mh:mw])

            # q^T/k^T per head, rows 64-127 zero-padded so every score
            # matmul is a full 128-contraction base-0 operand.
            qT = [pp.tile([P, S], BF16, tag=f"qT{h}", name=f"qT{h}")
                  for h in range(2)]
            kT = [pp.tile([P, S], BF16, tag=f"kT{h}", name=f"kT{h}")
                  for h in range(2)]
            for h in range(2):
                nc.gpsimd.memset(qT[h][D:P, :], 0.0)
                nc.gpsimd.memset(kT[h][D:P, :], 0.0)
            vT = pp.tile([P, S], BF16, tag="vT")

            # V_aug per (qtile, slice, head): [128k, 64+ones] stride-66
            VAUG = pp.tile([P, 2 * tot_sl * VA], BF16, tag="vaug")
            nc.gpsimd.memset(
                VAUG[:].rearrange("p (b va) -> p b va", va=VA)[:, :, D:D + 1],
                1.0)
            ON = pp.tile([P, S], BF16, tag="on")
            SEL = pp.tile([D + 1, D], BF16, tag="sel")
            nc.vector.memset(SEL[0:D, :], 0.0)
            nc.vector.memset(SEL[D:D + 1, :], 1.0)
            OTS = pp.tile([D + 1, NQT * 2 * P], BF16, tag="ots")

            # ---- Phase 1: QKV^T = W^T @ X^T, fp8 DoubleRow (2 contraction
            #      chunks per matmul); V_aug transposes interleaved as soon
            #      as the chunks covering each qtile's slices are ready ----
            xt3 = xt_sb[:].rearrange("p (c s) -> p c s", c=NC4)
            vt_done = 0
            with tc.tile_pool(name="ph1", bufs=4, space="PSUM") as ph1:

                def emit_vt(t0, t1):
                    # slices of qtiles t0..t1 are contiguous blocks in VAUG
                    blks = []
                    for t in range(t0, t1 + 1):
                        blks += slices[t]
                    nb = len(blks)
                    vt_ps = ph1.tile([P, nb * P], BF16, tag="vt",
                                     name="vt_ps")
                    for j, w in enumerate(blks):
                        nc.tensor.transpose(
                            out=vt_ps[:, j * P:(j + 1) * P],
                            in_=vT[:, w:w + P],
                            identity=ident[:],
                        )
                    vg0 = 2 * sl_base[t0] * VA
                    nc.vector.tensor_copy(
                        out=VAUG[:, vg0:vg0 + 2 * nb * VA].rearrange(
                            "p (b va) -> p b va", va=VA)[:, :, 0:D],
                        in_=vt_ps[:].rearrange(
                            "p (b d) -> p b d", d=D))

                for name in ("q", "v", "k"):
                    pss = [ph1.tile([P, QC], F32, tag="qkv",
                                    name="qkv_ps") for _ in range(NC4)]
                    for c in range(NC4):
                        for qc in range(NC4):
                            nc.tensor.matmul(
                                pss[qc][:],
                                lhsT=w_sb[name][:, c * P:(c + 1) * P],
                                rhs=xt3[:, c, qc * QC:(qc + 1) * QC],
                                start=(c == 0),
                                stop=(c == NC4 - 1),
                            )
                    for qc in range(NC4):
                        cs = slice(qc * QC, (qc + 1) * QC)
                        ps = pss[qc]
                        if name == "q":
                            for h in range(2):
                                hd = h * D
                                nc.scalar.activation(
                                    qT[h][0:D, cs], ps[hd:hd + D, :],
                                    mybir.ActivationFunctionType.Identity,
                                    bias=bqv_sb[hd:hd + D, 0:1],
                                )
                        elif name == "k":
                            for h in range(2):
                                hd = h * D
                                nc.vector.tensor_copy(
                                    out=kT[h][0:D, cs], in_=ps[hd:hd + D, :])
                        else:
                            nc.scalar.activation(
                                vT[:, cs], ps[:],
                                mybir.ActivationFunctionType.Identity,
                                bias=bqv_sb[:, 1:2],
                            )
                        if name == "v":
                            lim = (qc + 1) * QC
                            while (vt_done + 1 < NQT and
                                   slices[vt_done + 1][-1] + P <= lim):
                                emit_vt(vt_done, vt_done + 1)
                                vt_done += 2
                while vt_done < NQT:
                    t1 = min(vt_done + 1, NQT - 1)
                    emit_vt(vt_done, t1)
                    vt_done = t1 + 1

            # ---- Phase 2: banded attention, 3-stage skewed pipeline ----
            with tc.tile_pool(name="sp", bufs=3, space="PSUM") as sp, \
                 tc.tile_pool(name="otp", bufs=3, space="PSUM") as otp, \
                 tc.tile_pool(name="prp", bufs=2, space="PSUM") as prp, \
                 tc.tile_pool(name="pmp", bufs=3) as pmp, \
                 tc.tile_pool(name="obp", bufs=3) as obp:
                fstate = {}
                mstate = {}

                def stage_front(t):
                    """scores -> exp -> mask for qtile t"""
                    sl = slices[t]
                    ns = len(sl)
                    qs = slice(t * P, (t + 1) * P)
                    base = sl_base[t]
                    sc = sp.tile([P, 2 * ns * P], F32, tag="s", name="s_ps")
                    for h in range(2):
                        for j, w in enumerate(sl):
                            col = (ns * h + j) * P
                            nc.tensor.matmul(
                                sc[:, col:col + P],
                                lhsT=kT[h][0:D, w:w + P],
                                rhs=qT[h][0:D, qs],
                                start=True,
                                stop=True,
                            )
                    pm = pmp.tile([P, 2 * ns * P], BF16, tag="pm",
                                  name="pm_sb")
                    nc.scalar.activation(
                        pm[:], sc[:], mybir.ActivationFunctionType.Exp,
                        scale=SCALE)
                    mcol = base * P
                    mseg = msk_sb[:, mcol:mcol + ns * P]
                    eng = nc.vector if t % 2 == 0 else nc.gpsimd
                    eng.tensor_tensor(
                        out=pm[:].rearrange("p (h c) -> p h c", h=2),
                        in0=pm[:].rearrange("p (h c) -> p h c", h=2),
                        in1=mseg.rearrange("p (o c) -> p o c", o=1)
                            .to_broadcast([P, 2, ns * P]),
                        op=mybir.AluOpType.mult,
                    )
                    fstate[t] = pm

                def stage_mid(t):
                    """PV accumulate; per 2 qtiles: ot copy + den-replicate"""
                    sl = slices[t]
                    ns = len(sl)
                    pm = fstate.pop(t)
                    base = sl_base[t]
                    vg0 = 2 * base * VA
                    if t % 2 == 0:
                        ot = otp.tile([P, 4 * P], F32, tag="ot", name="ot_ps")
                        mstate[t] = ot
                    else:
                        ot = mstate[t - 1]
                    go = (t % 2) * 2 * P
                    for h in range(2):
                        oc = go + h * P
                        for j in range(ns):
                            va = vg0 + (2 * j + h) * VA
                            pc = (ns * h + j) * P
                            nc.tensor.matmul(
                                ot[0:D + 1, oc:oc + P],
                                lhsT=VAUG[:, va:va + D + 1],
                                rhs=pm[:, pc:pc + P],
                                start=(j == 0),
                                stop=(j == ns - 1),
                            )
                    if t % 2 == 1:
                        g = t // 2
                        oc0 = g * 4 * P
                        nc.scalar.copy(
                            out=OTS[:, oc0:oc0 + 4 * P], in_=ot[0:D + 1, :])
                        r2 = otp.tile([P, 4 * P], F32, tag="ot", name="r2")
                        nc.tensor.matmul(
                            r2[0:D, :],
                            lhsT=SEL[:],
                            rhs=OTS[:, oc0:oc0 + 4 * P],
                            start=True, stop=True,
                        )
                        mstate[t] = r2

                def stage_tail(todd):
                    """per 2 qtiles: recip + normalize + project + store"""
                    g = todd // 2
                    oc0 = g * 4 * P
                    del mstate[todd - 1]
                    r2 = mstate.pop(todd)
                    rr = pmp.tile([D, 4 * P], F32, tag="rr", name="rr")
                    nc.vector.reciprocal_approx_fast(
                        out=rr[:], in_=r2[0:D, :])
                    qs2 = slice(2 * g * P, (2 * g + 2) * P)
                    for h in range(2):
                        nc.vector.tensor_tensor(
                            out=ON[h * D:(h + 1) * D, qs2].rearrange(
                                "p (b c) -> p b c", b=2),
                            in0=OTS[0:D, oc0:oc0 + 4 * P].rearrange(
                                "p (b hh c) -> p b hh c", b=2, hh=2)[
                                :, :, h, :],
                            in1=rr[:].rearrange(
                                "p (b hh c) -> p b hh c", b=2, hh=2)[
                                :, :, h, :],
                            op=mybir.AluOpType.mult,
                        )
                    ob = obp.tile([P, 2 * DIM], BF16, tag="ob",
                                  name="ob_sb")
                    for i, t in enumerate((todd - 1, todd)):
                        qs = slice(t * P, (t + 1) * P)
                        pr = prp.tile([P, DIM], F32, tag="pr", name="pr_ps")
                        nc.tensor.matmul(
                            pr[:], lhsT=ON[:, qs], rhs=wo_sb,
                            start=True, stop=True,
                        )
                        if t % 2 == 0:
                            nc.scalar.copy(
                                out=ob[:, i * DIM:(i + 1) * DIM], in_=pr[:])
                        else:
                            nc.vector.tensor_copy(
                                out=ob[:, i * DIM:(i + 1) * DIM], in_=pr[:])
                    nc.sync.dma_start(
                        out=out_d[(todd - 1) * P:(todd + 1) * P, :]
                            .rearrange("(b p) c -> p b c", b=2),
                        in_=ob[:].rearrange("p (b c) -> p b c", b=2))

                stage_front(0)
                stage_front(1)
                for t in range(NQT):
                    if t + 2 < NQT:
                        stage_front(t + 2)
                    stage_mid(t)
                    if t % 2 == 1:
                        stage_tail(t)

    nc.compile()
    _CACHE[key] = nc
    return nc


def _pack(a):
    # [n*128, X] -> [128, n*X] grouping row-blocks along columns
    n = a.shape[0] // P
    return np.ascontiguousarray(
        a.reshape(n, P, a.shape[1]).transpose(1, 0, 2).reshape(P, -1))


def make_in_maps(x, routes, w_qkv, b_qkv, w_out):
    x = np.asarray(x, np.float32)
    routes = np.asarray(routes)
    w_qkv = np.asarray(w_qkv, np.float32)
    b_qkv = np.asarray(b_qkv, np.float32)
    w_out = np.asarray(w_out, np.float32)

    perm, inv, rk, slices = _plan_windows(routes)

    # count-mask blocks in permuted space: C~[k, q]
    Ct = np.zeros((S, S), np.float32)
    np.add.at(Ct, (rk.ravel(),
                   np.repeat(np.arange(S), KR)), 1.0)
    msk_cols = []
    for t, sl in enumerate(slices):
        for w in sl:
            msk_cols.append(Ct[w:w + P, t * P:(t + 1) * P])
    msk = np.ascontiguousarray(
        np.concatenate(msk_cols, axis=1).astype(NPBF16))

    xt = [_pack(np.ascontiguousarray(x[b][perm].T)).astype(NPBF16)
          for b in range(B)]

    in_maps = []
    for core in range(8):
        b = core // 4
        hp = core % 4
        col = hp * P
        wq = _pack(w_qkv[:, col:col + P])
        wk = _pack(w_qkv[:, DIM + col:DIM + col + P])
        wv = _pack(w_qkv[:, 2 * DIM + col:2 * DIM + col + P])
        w8 = np.concatenate([wq, wk, wv], axis=1).astype(NPBF16)
        wo = np.ascontiguousarray(w_out[col:col + P, :]).astype(NPBF16)
        bq = b_qkv[col:col + P].astype(np.float32)
        bv = b_qkv[2 * DIM + col:2 * DIM + col + P].astype(np.float32)
        bqv = np.stack([bq, bv], axis=1)
        in_maps.append(dict(xt=xt[b], w8=w8, wo=wo, bqv=bqv, msk=msk))
    return in_maps, perm, slices


def run(inputs, trace=False, trace_cores=None):
    in_maps, perm, slices = make_in_maps(
        inputs["x"], inputs["routes"], inputs["w_qkv"], inputs["b_qkv"],
        inputs["w_out"],
    )
    nc = build_nc(slices)
    res = run_bass_kernel_spmd(
        nc, in_maps, list(range(8)), trace=trace, trace_cores=trace_cores,
    )
    b_out = np.asarray(inputs["b_out"], np.float32)
    final = np.zeros((B, S, DIM), np.float32)
    for core in range(8):
        final[core // 4][perm] += np.asarray(
            res.results[core]["out"], np.float32)
    final += b_out[None, None, :]
    return final, res


def kernel(**inputs):
    final, _ = run(inputs, trace=False)
    return final
